# revision 20
# baseline (speedup 1.0000x reference)
"""Trainium2 Bass kernel for CohereAttention (QK-LayerNorm + interleaved RoPE +
GQA sliding-window attention), sharded over 8 NeuronCores.

Sharding: tensor-parallel over Q heads (4 per core); with H//KVH == 4 each core
owns exactly one KV head. Attention outputs are exchanged with an AllToAll
(token-major blocks) and o_proj is token-parallel: each core computes the full
4096-wide o_proj output for its 512-token slice, streaming the full wo.

Device-side layouts are transposed ([feature, token]) so every matmul contracts
over the partition axis at full PE rate:
  - QK-LayerNorm mean subtraction is folded into the projection weights on the
    host (subtract per-head column mean), leaving an RMS-style normalization.
  - RoPE rotate-half is a partition pair-swap (DVE stream_shuffle) with the sign
    folded into the sin table on the host.
  - Scores are computed transposed (S^T[j, q]); score chunks are paired two-to-
    a-PSUM-bank so one exp instruction covers 256 keys; sliding-window/causal
    masks are precomputed bf16 tiles applied with a DVE multiply; the softmax
    denominator is accumulated on DVE and reduced with one ones-matmul per
    query tile, then applied once per head at the drain.
"""

import sys

sys.path.insert(0, "/opt/trn_rl_repo")

import numpy as np
import ml_dtypes

import concourse.bass as bass
import concourse.mybir as mybir
import concourse.tile as tile
from concourse import bacc
from concourse.bass import ts, ds
from concourse.bass_utils import run_bass_kernel_spmd

B, S, H, KVH, D, HID = 2, 2048, 32, 8, 128, 4096
WINDOW = 512
EPS = 1e-5
SCALE = float(D) ** -0.5
NC = 8
HPC = H // NC              # q heads per core (4)
QW = HPC * D               # q width per core (512)
FCH = HID // 128           # contraction chunks (32)
TT = 512                   # projection token tile
QT = 256                   # attention query tile
NKC = (WINDOW + QT) // 128  # key chunks per query tile window (6)
TSL = S // NC              # tokens per (core, batch) slice for o_proj (256)

BF16 = mybir.dt.bfloat16
F32 = mybir.dt.float32
npbf16 = ml_dtypes.bfloat16

SWAP32 = [i ^ 1 for i in range(32)]  # adjacent-pair partition swap

_CACHE = {}


def _edge_masks():
    jj = np.arange(128)[:, None]
    qi = np.arange(QT)[None, :]

    def m(off):
        u = off + qi - jj
        return ((u >= 0) & (u < WINDOW)).astype(npbf16)

    mw = np.concatenate([m(512), m(384)], axis=1)   # chunks kk=0,1 (window edge)
    mc = np.concatenate([m(0), m(-128)], axis=1)    # chunks kk=4,5 (causal edge)
    return mw, mc


def _build_module():
    nc = bacc.Bacc(
        "TRN2",
        target_bir_lowering=False,
        debug=False,
        enable_asserts=False,
        num_devices=NC,
    )

    hT = nc.dram_tensor("hT", [B, HID, S], BF16, kind="ExternalInput").ap()
    cosT = nc.dram_tensor("cosT", [B, D, S], BF16, kind="ExternalInput").ap()
    sinT = nc.dram_tensor("sinT", [B, D, S], BF16, kind="ExternalInput").ap()
    wq = nc.dram_tensor("wq", [HID, QW], BF16, kind="ExternalInput").ap()
    wk = nc.dram_tensor("wk", [HID, D], BF16, kind="ExternalInput").ap()
    wv = nc.dram_tensor("wv", [HID, D], BF16, kind="ExternalInput").ap()
    wo = nc.dram_tensor("wo", [HID, HID], BF16, kind="ExternalInput").ap()
    winvq = nc.dram_tensor("winvq", [D, 1], BF16, kind="ExternalInput").ap()
    winvk = nc.dram_tensor("winvk", [D, 1], BF16, kind="ExternalInput").ap()
    out = nc.dram_tensor("out", [B, TSL, HID], F32, kind="ExternalOutput").ap()

    a2ain = [
        nc.dram_tensor(f"a2ain{b}", [NC, QW, TSL], BF16, kind="Internal").ap()
        for b in range(B)
    ]
    a2aout = [
        nc.dram_tensor(f"a2aout{b}", [NC, QW, TSL], BF16, kind="Internal").ap()
        for b in range(B)
    ]

    ident_d = nc.inline_tensor(np.eye(128, dtype=npbf16), name="ident").ap()
    ones_d = nc.inline_tensor(np.ones((128, 1), dtype=npbf16), name="onesv").ap()
    mw_np, mc_np = _edge_masks()
    maskw_d = nc.inline_tensor(mw_np, name="maskw").ap()
    maskc_d = nc.inline_tensor(mc_np, name="maskc").ap()

    rg = [list(range(NC))]

    with tile.TileContext(nc) as tc, \
            tc.tile_pool(name="sb", bufs=1) as sb, \
            tc.tile_pool(name="ps", bufs=1, space="PSUM") as ps:

        # --- resident weights / constants (wq chunked so proj starts early) ---
        # tag "wq" is double-buffered and later recycled for the streamed
        # o_proj weight slabs (same [128, FCH, 512] shape).
        wq_sb = sb.tile([128, FCH, QW], BF16, tag="wq", bufs=1, name="wq_sb")
        wq_r = wq.rearrange("(c p) n -> p c n", p=128)
        for f0 in range(0, FCH, 8):
            nc.sync.dma_start(wq_sb[:, ds(f0, 8), :], wq_r[:, ds(f0, 8), :])
        wk_sb = sb.tile([128, FCH, D], BF16, tag="wk", bufs=1, name="wk_sb")
        nc.sync.dma_start(wk_sb[:], wk.rearrange("(c p) n -> p c n", p=128))
        wv_sb = sb.tile([128, FCH, D], BF16, tag="wv", bufs=1, name="wv_sb")
        nc.sync.dma_start(wv_sb[:], wv.rearrange("(c p) n -> p c n", p=128))
        ident_sb = sb.tile([128, 128], BF16, tag="ident", bufs=1, name="ident_sb")
        nc.sync.dma_start(ident_sb[:], ident_d)
        ones_sb = sb.tile([128, 1], BF16, tag="ones", bufs=1, name="ones_sb")
        nc.sync.dma_start(ones_sb[:], ones_d)
        maskw_sb = sb.tile([128, 2 * QT], BF16, tag="maskw", bufs=1, name="maskw_sb")
        nc.sync.dma_start(maskw_sb[:], maskw_d)
        maskc_sb = sb.tile([128, 2 * QT], BF16, tag="maskc", bufs=1, name="maskc_sb")
        nc.sync.dma_start(maskc_sb[:], maskc_d)
        winvq_sb = sb.tile([D, 1], BF16, tag="winvq", bufs=1, name="winvq_sb")
        nc.sync.dma_start(winvq_sb[:], winvq)
        winvk_sb = sb.tile([D, 1], BF16, tag="winvk", bufs=1, name="winvk_sb")
        nc.sync.dma_start(winvk_sb[:], winvk)
        eps_sb = sb.tile([1, 1], F32, tag="eps", bufs=1, name="eps_sb")
        nc.vector.memset(eps_sb[:], EPS)

        def ln_rope(qps, winv_sb, cos_sb, sin_sb, tt, dst):
            """LayerNorm (mean pre-folded) + interleaved RoPE on a transposed
            [d, TT] psum tile; writes bf16 into dst[:, tt*TT:...]."""
            sq = sb.tile([128, TT], BF16, tag="sq", bufs=1, name="sq")
            nc.scalar.square(sq[:], qps[:])
            qsb = sb.tile([128, TT], BF16, tag="qsb", bufs=1, name="qsb")
            nc.scalar.copy(qsb[:], qps[:])  # frees the psum bank early
            ssq = ps.tile([1, TT], F32, tag="misc", bufs=2, name="ssq")
            nc.tensor.matmul(ssq[:], winv_sb[:], sq[:], start=True, stop=True)
            std = sb.tile([1, TT], F32, tag="std", bufs=3, name="std")
            nc.scalar.activation(
                std[:], ssq[:], mybir.ActivationFunctionType.Sqrt,
                bias=eps_sb[:], scale=1.0 / D,
            )
            rstd = sb.tile([1, TT], F32, tag="rstd", bufs=3, name="rstd")
            nc.vector.reciprocal(rstd[:], std[:])
            rbc = sb.tile([128, TT], F32, tag="rbc", bufs=1, name="rbc")
            nc.gpsimd.partition_broadcast(rbc[:], rstd[:])
            qn = sb.tile([128, TT], BF16, tag="qn", bufs=2, name="qn")
            nc.vector.tensor_mul(qn[:], qsb[:], rbc[:])
            qs = sb.tile([128, TT], BF16, tag="qs", bufs=2, name="qs")
            nc.vector.stream_shuffle(qs[:], qn[:], SWAP32)
            t1 = sb.tile([128, TT], BF16, tag="t1", bufs=2, name="t1")
            nc.vector.tensor_mul(t1[:], qn[:], cos_sb[:, ts(tt, TT)])
            t2 = sb.tile([128, TT], BF16, tag="t2", bufs=2, name="t2")
            nc.vector.tensor_mul(t2[:], qs[:], sin_sb[:, ts(tt, TT)])
            nc.vector.tensor_add(dst[:, ts(tt, TT)], t1[:], t2[:])

        qT = {}   # (b, h) -> [128, S] bf16 rope'd normalized q, transposed
        kT = {}   # b -> [128, S]
        Vn = {}   # b -> [128, S] (natural [j, d] in 128-col chunks)
        vT = {}   # b -> [128, S] transposed v (pre PE-transpose)
        trig = {}  # b -> (cos_sb, sin_sb)

        def proj_setup(b):
            cos_sb = sb.tile([128, S], BF16, tag="cos", bufs=1, name="cos_sb")
            nc.sync.dma_start(cos_sb[:], cosT[b])
            sin_sb = sb.tile([128, S], BF16, tag="sin", bufs=1, name="sin_sb")
            nc.sync.dma_start(sin_sb[:], sinT[b])
            trig[b] = (cos_sb, sin_sb)
            for h in range(HPC):
                qT[(b, h)] = sb.tile([128, S], BF16, tag="qT", bufs=4,
                                     name=f"qT{b}{h}")
            kT[b] = sb.tile([128, S], BF16, tag="kT", bufs=2, name=f"kT{b}")
            vT[b] = sb.tile([128, S], BF16, tag="vT", bufs=1, name=f"vT{b}")
            Vn[b] = sb.tile([128, S], BF16, tag="Vn", bufs=1, name=f"Vn{b}")

        def proj_tt(b, tt):
            with nc.named_scope(f"proj_b{b}"):
                cos_sb, sin_sb = trig[b]
                qps = [
                    ps.tile([128, TT], F32, tag=f"acc{i}", bufs=1,
                            name=f"qps{i}")
                    for i in range(HPC)
                ]
                kps = ps.tile([128, TT], F32, tag="acck", bufs=1, name="kps")
                vps = ps.tile([128, TT], F32, tag="accv", bufs=1, name="vps")
                for f in range(FCH):
                    ht_t = sb.tile([128, TT], BF16, tag="ht", bufs=3,
                                   name="ht_t")
                    nc.sync.dma_start(
                        ht_t[:], hT[b, ds(f * 128, 128), ts(tt, TT)]
                    )
                    st = f == 0
                    sp = f == FCH - 1
                    for h in range(HPC):
                        nc.tensor.matmul(
                            qps[h][:], wq_sb[:, f, ts(h, D)], ht_t[:],
                            start=st, stop=sp,
                        )
                    nc.tensor.matmul(kps[:], wk_sb[:, f, :], ht_t[:],
                                     start=st, stop=sp)
                    nc.tensor.matmul(vps[:], wv_sb[:, f, :], ht_t[:],
                                     start=st, stop=sp)
                for h in range(HPC):
                    ln_rope(qps[h], winvq_sb, cos_sb, sin_sb, tt, qT[(b, h)])
                ln_rope(kps, winvk_sb, cos_sb, sin_sb, tt, kT[b])
                nc.scalar.copy(vT[b][:, ts(tt, TT)], vps[:])

        def proj_vtrans(b):
            # transpose v to natural [j, d] layout for the PV matmul
            with nc.named_scope(f"proj_b{b}"):
                for j in range(S // 128):
                    tp = ps.tile([128, 128], BF16, tag="misc", bufs=2, name="tp")
                    nc.tensor.transpose(tp[:], vT[b][:, ts(j, 128)], ident_sb[:])
                    nc.scalar.copy(Vn[b][:, ts(j, 128)], tp[:])

        def attn_head(b, h):
            with nc.named_scope(f"attn_b{b}"):
                attn_sb = sb.tile([128, S], BF16, tag="attn", bufs=2,
                                  name="attn_sb")
                lden = sb.tile([1, S], BF16, tag="lden", bufs=2, name="lden")
                for qt in range(S // QT):
                    i0 = qt * QT
                    kstart = max(0, (WINDOW - i0) // 128)
                    npair = (NKC - kstart) // 2
                    ops = ps.tile([128, QT], F32,
                                  tag="acc3" if qt % 2 == 0 else "acck",
                                  bufs=1, name="ops")
                    acc = sb.tile([128, QT], BF16, tag="lacc", bufs=1,
                                  name="lacc")
                    for pp in range(npair):
                        kk = kstart + 2 * pp
                        j0 = i0 - WINDOW + kk * 128
                        sps = ps.tile([128, 2 * QT], F32,
                                      tag=f"acc{pp % 3}", bufs=1, name="sps")
                        nc.tensor.matmul(
                            sps[:, 0:QT], kT[b][:, ds(j0, 128)],
                            qT[(b, h)][:, ds(i0, QT)],
                            start=True, stop=True,
                        )
                        nc.tensor.matmul(
                            sps[:, QT:2 * QT], kT[b][:, ds(j0 + 128, 128)],
                            qT[(b, h)][:, ds(i0, QT)],
                            start=True, stop=True,
                        )
                        pt = sb.tile([128, 2 * QT], BF16, tag="pt", bufs=2,
                                     name="pt")
                        nc.scalar.activation(
                            pt[:], sps[:], mybir.ActivationFunctionType.Exp,
                            scale=SCALE,
                        )
                        if kk == 0:  # window edge pair (kk=0,1)
                            nc.vector.tensor_mul(pt[:], pt[:], maskw_sb[:])
                        elif kk == 4:  # causal edge pair (kk=4,5)
                            nc.vector.tensor_mul(pt[:], pt[:], maskc_sb[:])
                        first = pp == 0
                        last = pp == npair - 1
                        nc.tensor.matmul(
                            ops[:], Vn[b][:, ds(j0, 128)], pt[:, 0:QT],
                            start=first, stop=False,
                        )
                        nc.tensor.matmul(
                            ops[:], Vn[b][:, ds(j0 + 128, 128)],
                            pt[:, QT:2 * QT],
                            start=False, stop=last,
                        )
                        # bf16 running sum of exp() is plenty for the softmax
                        # denominator (rel tol here is 2e-2)
                        with nc.allow_low_precision(reason="softmax denom"):
                            if first:
                                nc.vector.tensor_add(acc[:], pt[:, 0:QT],
                                                     pt[:, QT:2 * QT])
                            else:
                                nc.vector.tensor_add(acc[:], acc[:],
                                                     pt[:, 0:QT])
                                nc.vector.tensor_add(acc[:], acc[:],
                                                     pt[:, QT:2 * QT])
                    lps = ps.tile([1, QT], F32, tag="misc", bufs=2, name="lps")
                    nc.tensor.matmul(lps[:], ones_sb[:], acc[:],
                                     start=True, stop=True)
                    nc.vector.tensor_copy(lden[:, ds(i0, QT)], lps[:])
                    nc.vector.tensor_copy(attn_sb[:, ds(i0, QT)], ops[:])
                # batched normalization: one reciprocal/broadcast/mul per head
                linv = sb.tile([1, S], BF16, tag="linv", bufs=2, name="linv")
                with nc.allow_low_precision(reason="softmax denom"):
                    nc.vector.reciprocal(linv[:], lden[:])
                lbc = sb.tile([128, S], BF16, tag="lbc", bufs=1, name="lbc")
                nc.gpsimd.partition_broadcast(lbc[:], linv[:])
                nc.vector.tensor_mul(attn_sb[:], attn_sb[:], lbc[:])
                for n in range(NC):
                    nc.sync.dma_start(
                        a2ain[b][n, ts(h, 128), :],
                        attn_sb[:, ts(n, TSL)],
                    )

        def a2a_phase(b):
            nc.gpsimd.collective_compute(
                "AllToAll",
                mybir.AluOpType.bypass,
                replica_groups=rg,
                ins=[a2ain[b][:]],
                outs=[a2aout[b][:]],
            )

        def oproj_phase():
            # token-parallel o_proj: full wo streamed once; out[b, t, :] for
            # this core's TSL-token slice of each batch.
            with nc.named_scope("oproj"):
                af = {}
                for b in range(B):
                    af[b] = sb.tile([128, NC, HPC, TSL], BF16, tag="af",
                                    bufs=2, name=f"af{b}")
                    nc.sync.dma_start(
                        af[b][:],
                        a2aout[b].rearrange("n (c p) t -> p n c t", p=128),
                    )
                wo_r = wo.rearrange("(c p) n -> p c n", p=128)
                for ob in range(HID // 512):
                    # stream this 512-wide output slab's weights in 8-chunk
                    # pieces; run the four 32-chunk accumulation chains
                    # (b, t2) interleaved so each piece is consumed in order.
                    wo_p = [
                        sb.tile([128, 8, 512], BF16, tag="wo", bufs=2,
                                name="wo_p")
                        for _ in range(FCH // 8)
                    ]
                    for i, wp in enumerate(wo_p):
                        nc.sync.dma_start(
                            wp[:], wo_r[:, ds(8 * i, 8), ts(ob, 512)]
                        )
                    po = {}
                    for b in range(B):
                        for t2 in range(TSL // 128):
                            po[(b, t2)] = ps.tile(
                                [128, 512], F32, tag=f"acc{2 * b + t2}",
                                bufs=1, name="po",
                            )
                    for c in range(FCH):
                        for b in range(B):
                            for t2 in range(TSL // 128):
                                nc.tensor.matmul(
                                    po[(b, t2)][:],
                                    af[b][:, c // HPC, c % HPC, ts(t2, 128)],
                                    wo_p[c // 8][:, c % 8, :],
                                    start=(c == 0), stop=(c == FCH - 1),
                                )
                    for b in range(B):
                        for t2 in range(TSL // 128):
                            ot = sb.tile([128, 512], F32, tag="ot", bufs=2,
                                         name="ot")
                            nc.scalar.copy(ot[:], po[(b, t2)][:])
                            nc.sync.dma_start(
                                out[b, ds(t2 * 128, 128), ts(ob, 512)], ot[:]
                            )

        proj_setup(0)
        for tt in range(S // TT):
            proj_tt(0, tt)
        proj_vtrans(0)
        for h in range(HPC):
            attn_head(0, h)
        a2a_phase(0)
        proj_setup(1)
        for tt in range(S // TT):
            proj_tt(1, tt)
        proj_vtrans(1)
        for h in range(HPC):
            attn_head(1, h)
        a2a_phase(1)
        oproj_phase()

    nc.compile()
    return nc


def _prep_inputs(inputs):
    hidden = np.asarray(inputs["hidden_states"], np.float32)
    pos = np.asarray(inputs["position_ids"])
    cos = np.asarray(inputs["cos"], np.float32)
    sin = np.asarray(inputs["sin"], np.float32)
    wq = np.asarray(inputs["wq"], np.float32)
    wk = np.asarray(inputs["wk"], np.float32)
    wv = np.asarray(inputs["wv"], np.float32)
    wo = np.asarray(inputs["wo"], np.float32)
    qw = np.asarray(inputs["q_norm_w"], np.float32)
    kw = np.asarray(inputs["k_norm_w"], np.float32)

    hT = np.ascontiguousarray(hidden.transpose(0, 2, 1)).astype(npbf16)
    cosT = np.ascontiguousarray(cos[pos].transpose(0, 2, 1)).astype(npbf16)
    sinT_f = sin[pos].transpose(0, 2, 1).copy()
    sinT_f[:, 0::2, :] *= -1.0
    sinT = np.ascontiguousarray(sinT_f).astype(npbf16)

    winvq = (1.0 / np.where(qw == 0, 1, qw) ** 2).astype(npbf16).reshape(D, 1)
    winvk = (1.0 / np.where(kw == 0, 1, kw) ** 2).astype(npbf16).reshape(D, 1)
    wo_full = np.ascontiguousarray(wo).astype(npbf16)

    in_maps = []
    for c in range(NC):
        wq_c = wq[:, c * QW:(c + 1) * QW].copy()
        for j in range(HPC):
            blk = wq_c[:, j * D:(j + 1) * D]
            blk -= blk.mean(axis=1, keepdims=True)
            blk *= qw[None, :]
        wk_c = wk[:, c * D:(c + 1) * D].copy()
        wk_c -= wk_c.mean(axis=1, keepdims=True)
        wk_c *= kw[None, :]
        in_maps.append({
            "hT": hT,
            "cosT": cosT,
            "sinT": sinT,
            "wq": np.ascontiguousarray(wq_c).astype(npbf16),
            "wk": np.ascontiguousarray(wk_c).astype(npbf16),
            "wv": np.ascontiguousarray(wv[:, c * D:(c + 1) * D]).astype(npbf16),
            "wo": wo_full,
            "winvq": winvq,
            "winvk": winvk,
        })
    return in_maps


def _run(inputs, **kwargs):
    if "nc" not in _CACHE:
        _CACHE["nc"] = _build_module()
    nc = _CACHE["nc"]
    in_maps = _prep_inputs(inputs)
    res = run_bass_kernel_spmd(nc, in_maps, core_ids=list(range(NC)), **kwargs)
    # core c holds out[b, c*TSL:(c+1)*TSL, :] for each batch
    shards = [res.results[c]["out"].reshape(B, TSL, HID) for c in range(NC)]
    full = np.concatenate(shards, axis=1).astype(np.float32)
    return full, res


def kernel(**inputs) -> np.ndarray:
    out, _ = _run(inputs)
    return out


if __name__ == "__main__":
    import reference
    ins = {k: np.asarray(v) for k, v in reference.setup_inputs().items()}
    expected = np.asarray(reference.reference(**reference.setup_inputs()))
    actual = kernel(**ins)
    err = np.linalg.norm(actual - expected) / np.linalg.norm(expected)
    print("Relative error:", err)
